# revision 1
# baseline (speedup 1.0000x reference)
"""Causal single-head attention on 8 Trainium2 NeuronCores.

Problem: x [4, 2048, 1024] fp32; Wq/Wk/Wv [1024, 1024] fp32.
  q/k/v = x @ W*; scores = q k^T / 32 (causal); out = softmax(scores) @ v.

Sharding: 8 cores = 4 batches x 2 roles. Within a batch, the 16
128-row q-blocks are split alternately: role r takes global blocks
g = 2j+r (j = 0..7) — this balances causal attention work between the
pair. Every core projects K~ = x @ (Wk Wq^T) for the full 2048 kv
tokens of its batch and runs causal attention over a padded kv prefix
of 2j+2 blocks per q-block. Both other projections are algebraically
folded away: scores = x_q (Wq Wk^T) x_kv^T, so raw x_q columns serve
directly as Q^T (no Q matmuls); and out = (attn @ x_kv) @ Wv, so the
attn@V contraction runs against raw x rows and Wv is applied to the
[1024, 1024] normalized context of this core's own q rows afterwards
(128 matmuls instead of a 256-matmul V projection of all kv tokens).
Each q-block's tail (normalize -> PE-transpose -> @Wv -> store) is
emitted one block late, software-pipelined under the next block's
score/context matmuls.

The program is SPMD-uniform: role differences live only in the
host-gathered inputs (xq = q-token columns of x^T in q-block order;
xt = full x^T) and in the [128, 256] mask applied to the last two kv
blocks of each padded row ([tril|zeros] for role 0, [ones|tril] for
role 1).

Numerics: all matmuls in bf16 (inputs rounded on host) with fp32
PSUM accumulation; softmax in fp32 without max-subtraction (scores
are O(5), exp can't overflow), normalization deferred to after the
attn@V matmul. End-to-end max-abs error vs the fp32 reference is
~6e-3 of the output scale.
"""

import numpy as np
import ml_dtypes

import concourse.bass as bass
import concourse.bacc as bacc
import concourse.tile as tile
from concourse import mybir
from concourse.bass_utils import run_bass_kernel_spmd
from concourse.masks import make_identity

P = 128
D = 1024          # d_in
E = 1024          # d_out
T = 2048          # seq len
B = 4             # batch
DT = D // P       # 8 d-tiles
ET = E // P       # 8 e-tiles
QB = 8            # q blocks per core
KVB = T // P      # 16 kv blocks
NCORES = 8

FP32 = mybir.dt.float32
BF16 = mybir.dt.bfloat16

_CACHED_NC = None


def _build(do_kv=True, do_attn=True, pmm_bufs=2, pt_bufs=2, pu_bufs=2, work_bufs=2, x_bufs=3):
    nc = bacc.Bacc(None, target_bir_lowering=False)
    # xq: x^T columns of our q tokens (raw features = Q side of the folded
    # score matmul). wk here is the host-folded Wk @ Wq^T.
    xq = nc.dram_tensor("xq", [D, QB * P], BF16, kind="ExternalInput")
    xt = nc.dram_tensor("xt", [D, T], BF16, kind="ExternalInput")
    xn = nc.dram_tensor("xn", [T, D], BF16, kind="ExternalInput")
    wk = nc.dram_tensor("wk", [D, E], BF16, kind="ExternalInput")
    wv = nc.dram_tensor("wv", [D, E], BF16, kind="ExternalInput")
    mask = nc.dram_tensor("mask", [P, 2 * P], BF16, kind="ExternalInput")
    out = nc.dram_tensor("out", [QB * P, E], FP32, kind="ExternalOutput")

    xq_r = xq.rearrange("(dt p) t -> p dt t", p=P)
    xt_r = xt.rearrange("(dt p) t -> p dt t", p=P)

    with tile.TileContext(nc) as tc:
        with (
            tc.tile_pool(name="const", bufs=1) as const,
            tc.tile_pool(name="big", bufs=1) as big,
            tc.tile_pool(name="wpool", bufs=1) as wpool,
            tc.tile_pool(name="xpool", bufs=x_bufs) as xpool,
            tc.tile_pool(name="work", bufs=work_bufs) as work,
            tc.tile_pool(name="small", bufs=8) as small,
            tc.tile_pool(name="pmm", bufs=pmm_bufs, space="PSUM") as pmm,
            tc.tile_pool(name="pt", bufs=pt_bufs, space="PSUM") as pt,
            tc.tile_pool(name="pu", bufs=pu_bufs, space="PSUM") as pu,
        ):
            ident = const.tile([P, P], BF16)
            make_identity(nc, ident[:])
            mask_sb = const.tile([P, 2 * P], BF16)
            nc.sync.dma_start(out=mask_sb[:], in_=mask[:, :])

            KT = big.tile([P, ET, T], BF16)       # K~^T, e-major
            XN = big.tile([P, KVB, D], BF16)      # raw x rows, kv-tile major
            QT = big.tile([P, ET, QB * P], BF16)  # Q^T for our 1024 q rows
            nc.sync.dma_start(out=XN[:], in_=xn.rearrange("(tt p) d -> p tt d", p=P))

            wk_sb = wpool.tile([P, DT, E], BF16, tag="wk")
            nc.sync.dma_start(out=wk_sb[:], in_=wk.rearrange("(dt p) e -> p dt e", p=P))
            wv_sb = wpool.tile([P, DT, E], BF16, tag="wv")
            nc.sync.dma_start(out=wv_sb[:], in_=wv.rearrange("(dt p) e -> p dt e", p=P))

            # Q^T is just the raw q-token features, DMA'd straight in
            nc.sync.dma_start(out=QT[:], in_=xq_r[:, :, :])

            # ---- Phase A: K^T and V projections over the full 2048 kv tokens
            for c in range(T // 512 if do_kv else 0):
                xc = xpool.tile([P, DT, 512], BF16, tag="x")
                nc.sync.dma_start(out=xc[:], in_=xt_r[:, :, 512 * c:512 * (c + 1)])
                for e in range(ET):
                    ps = pmm.tile([P, 512], FP32, tag="mm")
                    for dt in range(DT):
                        nc.tensor.matmul(ps[:], wk_sb[:, dt, e * P:(e + 1) * P],
                                         xc[:, dt, :],
                                         start=(dt == 0), stop=(dt == DT - 1))
                    nc.scalar.copy(KT[:, e, 512 * c:512 * (c + 1)], ps[:])

            # ---- Phase C: attention per q block.
            # The per-block tail (normalize -> transpose -> @Wv -> store) is
            # emitted one block late so its DVE/ACT dependencies resolve
            # while the PE runs the next block's score/context matmuls.
            def emit_tail(U, sums, j):
                recip = small.tile([P, 1], FP32)
                nc.vector.reciprocal(recip[:], sums[:])
                c_sb = work.tile([P, D], BF16, tag="csb")
                for dh in range(2):
                    nc.vector.tensor_scalar_mul(c_sb[:, dh * 512:(dh + 1) * 512],
                                                U[:, dh * 512:(dh + 1) * 512],
                                                recip[:])
                ps_c = pt.tile([P, D], BF16, tag="pt")
                for i in range(DT):
                    nc.tensor.transpose(ps_c[:, i * P:(i + 1) * P],
                                        c_sb[:, i * P:(i + 1) * P], ident[:])
                ct_sb = work.tile([P, D], BF16, tag="ct")
                nc.scalar.copy(ct_sb[:], ps_c[:])
                out_sb = work.tile([P, E], FP32, tag="out")
                for eh in range(2):
                    ps_o = pmm.tile([P, 512], FP32, tag="mm")
                    for dt in range(DT):
                        nc.tensor.matmul(ps_o[:], ct_sb[:, dt * P:(dt + 1) * P],
                                         wv_sb[:, dt, eh * 512:(eh + 1) * 512],
                                         start=(dt == 0), stop=(dt == DT - 1))
                    nc.scalar.copy(out_sb[:, eh * 512:(eh + 1) * 512], ps_o[:])
                nc.sync.dma_start(out=out[j * P:(j + 1) * P, :], in_=out_sb[:])

            pending = None
            for j in range(QB if do_attn else 0):
                n_kb = 2 * j + 2          # padded kv blocks for this q block
                widths = [512] * ((j + 1) // 2) + ([256] if j % 2 == 0 else [])
                sums = small.tile([P, 1], FP32)
                nc.vector.memset(sums[:], 0.0)
                U = pu.tile([P, E], FP32, tag="pu")
                c0 = 0
                for ci, w in enumerate(widths):
                    last = (ci == len(widths) - 1)
                    ps_s = pmm.tile([P, 512], FP32, tag="mm")
                    for et in range(ET):
                        nc.tensor.matmul(ps_s[:, :w], QT[:, et, j * P:(j + 1) * P],
                                         KT[:, et, c0:c0 + w],
                                         start=(et == 0), stop=(et == ET - 1))
                    exps = work.tile([P, 512], BF16, tag="exps")
                    nc.scalar.activation(exps[:, :w], ps_s[:, :w],
                                         mybir.ActivationFunctionType.Exp,
                                         scale=1.0 / 32.0)
                    if last:
                        nc.vector.tensor_mul(exps[:, w - 256:w],
                                             exps[:, w - 256:w], mask_sb[:])
                    csum = small.tile([P, 1], FP32)
                    nc.vector.tensor_reduce(csum[:], exps[:, :w],
                                            axis=mybir.AxisListType.X,
                                            op=mybir.AluOpType.add)
                    nc.vector.tensor_add(sums[:], sums[:], csum[:])

                    ps_t = pt.tile([P, 512], BF16, tag="pt")
                    nblk = w // P
                    for i in range(nblk):
                        nc.tensor.transpose(ps_t[:, i * P:(i + 1) * P],
                                            exps[:, i * P:(i + 1) * P], ident[:])
                    expsT = work.tile([P, 512], BF16, tag="expsT")
                    nc.scalar.copy(expsT[:, :w], ps_t[:, :w])
                    for i in range(nblk):
                        kb = c0 // P + i
                        for dh in range(2):
                            nc.tensor.matmul(U[:, dh * 512:(dh + 1) * 512],
                                             expsT[:, i * P:(i + 1) * P],
                                             XN[:, kb, dh * 512:(dh + 1) * 512],
                                             start=(kb == 0), stop=(kb == n_kb - 1))
                    c0 += w
                if pending is not None:
                    emit_tail(*pending)
                pending = (U, sums, j)
            if pending is not None:
                emit_tail(*pending)

    nc.compile()
    return nc


def _get_nc():
    global _CACHED_NC
    if _CACHED_NC is None:
        _CACHED_NC = _build()
    return _CACHED_NC


def _prep_inputs(x, Wq, Wk, Wv):
    bf = ml_dtypes.bfloat16
    tril = np.tril(np.ones((P, P), np.float32))
    ones = np.ones((P, P), np.float32)
    zeros = np.zeros((P, P), np.float32)
    # fold Wq into the K projection: scores = x_q (Wq Wk^T) x_kv^T, so the
    # kernel projects K~ = x @ (Wk Wq^T) and uses raw x_q as Q.
    wfold = (np.asarray(Wk, np.float64) @ np.asarray(Wq, np.float64).T)
    wk_b = wfold.astype(np.float32).astype(bf)
    wv_b = np.asarray(Wv, np.float32).astype(bf)
    in_maps = []
    for core in range(NCORES):
        b, r = core // 2, core % 2
        xt = np.ascontiguousarray(x[b].T.astype(np.float32)).astype(bf)
        xqc = np.ascontiguousarray(
            xt.reshape(D, KVB, P)[:, r::2, :].reshape(D, QB * P))
        m = (np.concatenate([tril, zeros], axis=1) if r == 0
             else np.concatenate([ones, tril], axis=1)).astype(bf)
        in_maps.append({
            "xq": xqc,
            "xt": xt,
            "xn": np.ascontiguousarray(x[b].astype(np.float32)).astype(bf),
            "wk": wk_b,
            "wv": wv_b,
            "mask": m,
        })
    return in_maps


def _assemble(results, x_shape):
    outp = np.empty(x_shape, np.float32)
    for core in range(NCORES):
        b, r = core // 2, core % 2
        co = results[core]["out"]
        for j in range(QB):
            g = 2 * j + r
            outp[b, g * P:(g + 1) * P, :] = co[j * P:(j + 1) * P, :]
    return outp


def kernel(x, Wq, Wk, Wv):
    assert x.shape == (B, T, D) and Wq.shape == (D, E)
    nc = _get_nc()
    in_maps = _prep_inputs(x, Wq, Wk, Wv)
    res = run_bass_kernel_spmd(nc, in_maps, core_ids=list(range(NCORES)))
    return _assemble(res.results, x.shape)



# revision 9
# speedup vs baseline: 1.7674x; 1.7674x over previous
"""Causal single-head attention on 8 Trainium2 NeuronCores — fp8 DoubleRow.

Problem: x [4, 2048, 1024] fp32; Wq/Wk/Wv [1024, 1024] fp32.
  q/k/v = x @ W*; scores = q k^T / 32 (causal); out = softmax(scores) @ v.

Sharding: 8 cores = 4 batches x 2 roles; role r owns global q-blocks
g = 2j+r (j = 0..7), padded kv window of 2j+2 blocks per q-block
(role-specific masks make the programs SPMD-uniform).

Algorithm (per core): fold Wq into the Q side: Qt = x_q @ (32 Wq Wk^T), so
scores^T = (xt^T)^T ... concretely every matmul contracts over the SBUF
partition dim with fp8 DoubleRow pairs (K=256/instr at 0.5 cyc/row):
  A) Qt[d',q] accumulated from (xq_hi+lo) x (wf_hi+lo), 3 cross terms,
     then split on device into Qt_hi/lo fp8.
  B) per q-block j: scoresT[kv,q] = (xt_hi+lo)^T (Qt_hi+lo), 3 terms;
     exps = exp(scores/1024 - c_j) written straight to fp8 (c_j a
     per-block constant shift keeping exps and U in fp8 range);
     causal/padding masks applied multiplicatively on the fp8 exps;
     sums via a DoubleRow ones(=32)-matmul; ctxT[d,q] accumulated from
     (xn_hi+lo) x exps_fp8, 2 terms.
  C) U = ctxT split into U_hi/lo fp8; out[q,e] = (U_hi+lo)^T (wv_hi+lo),
     3 terms; normalized by recip = 1/(32*sums) fused into the psum->sbuf
     copy. Block j=0 (windows 1..256) runs its exps/ctx/Wv in bf16 to
     dodge fp8 dynamic-range limits on tiny softmax sums.

All splits of host tensors are done on host (hi = fp8(a), lo = fp8(a-hi));
weights are pre-scaled by 32 so their hi/lo split stays in fp8 normal
range.  Measured end-to-end rel err ~7e-3 vs the fp32 reference.
"""

import numpy as np
import ml_dtypes

import concourse.bass as bass
import concourse.bacc as bacc
import concourse.tile as tile
from concourse import mybir
from concourse.bass_utils import run_bass_kernel_spmd
from concourse.masks import make_identity

P = 128
D = 1024
T = 2048
B = 4
DT = D // P       # 8 d tiles
QB = 8            # q blocks per core
KVB = T // P      # 16 kv blocks
NCORES = 8

FP32 = mybir.dt.float32
BF16 = mybir.dt.bfloat16
FP8 = mybir.dt.float8e4
DR = mybir.MatmulPerfMode.DoubleRow
F8 = ml_dtypes.float8_e4m3
BF = ml_dtypes.bfloat16

_CACHED_NC = None


def _build():
    nc = bacc.Bacc(None, target_bir_lowering=False)
    xq_hi = nc.dram_tensor("xq_hi", [P, DT, QB * P], FP8, kind="ExternalInput")
    xq_lo = nc.dram_tensor("xq_lo", [P, DT, QB * P], FP8, kind="ExternalInput")
    xt_hi = nc.dram_tensor("xt_hi", [P, DT, T], FP8, kind="ExternalInput")
    xt_lo = nc.dram_tensor("xt_lo", [P, DT, T], FP8, kind="ExternalInput")
    xn_hi = nc.dram_tensor("xn_hi", [P, KVB, D], FP8, kind="ExternalInput")
    xn_lo = nc.dram_tensor("xn_lo", [P, KVB, D], FP8, kind="ExternalInput")
    xn_bf = nc.dram_tensor("xn_bf", [P, 2, D], BF16, kind="ExternalInput")
    wf_hi = nc.dram_tensor("wf_hi", [P, DT, D], FP8, kind="ExternalInput")
    wf_lo = nc.dram_tensor("wf_lo", [P, DT, D], FP8, kind="ExternalInput")
    wv_hi = nc.dram_tensor("wv_hi", [P, DT, D], FP8, kind="ExternalInput")
    wv_lo = nc.dram_tensor("wv_lo", [P, DT, D], FP8, kind="ExternalInput")
    wv_bf = nc.dram_tensor("wv_bf", [P, DT, D], BF16, kind="ExternalInput")
    mask8 = nc.dram_tensor("mask8", [P, 2 * P], FP8, kind="ExternalInput")
    maskb = nc.dram_tensor("maskb", [P, 2 * P], BF16, kind="ExternalInput")
    cbias = nc.dram_tensor("cbias", [P, QB], FP32, kind="ExternalInput")
    out = nc.dram_tensor("out", [QB * P, D], FP32, kind="ExternalOutput")

    with tile.TileContext(nc) as tc:
        with (
            tc.tile_pool(name="const", bufs=1) as const,
            tc.tile_pool(name="big", bufs=1) as big,
            tc.tile_pool(name="exps", bufs=2) as epool,
            tc.tile_pool(name="upool", bufs=2) as upool,
            tc.tile_pool(name="opool", bufs=2) as opool,
            tc.tile_pool(name="small", bufs=8) as small,
            tc.tile_pool(name="psA", bufs=2, space="PSUM") as psA,
            tc.tile_pool(name="psSum", bufs=1, space="PSUM") as psSum,
            tc.tile_pool(name="psU", bufs=2, space="PSUM") as psU,
            tc.tile_pool(name="psO", bufs=1, space="PSUM") as psO,
        ):
            identf = const.tile([1, 1], FP32)
            nc.vector.memset(identf[:], 1.0)
            ones8 = const.tile([P, 2, P], FP8)
            nc.vector.memset(ones8[:], 32.0)
            onesb = const.tile([P, P], BF16)
            nc.vector.memset(onesb[:], 32.0)
            msk8 = const.tile([P, 2 * P], FP8)
            nc.sync.dma_start(out=msk8[:], in_=mask8[:, :])
            mskb = const.tile([P, 2 * P], BF16)
            nc.sync.dma_start(out=mskb[:], in_=maskb[:, :])
            cb = const.tile([P, QB], FP32)
            nc.sync.dma_start(out=cb[:], in_=cbias[:, :])

            WF = [big.tile([P, DT, D], FP8, tag=f"wf{s}", name=f"WF{s}") for s in range(2)]
            XQ = [big.tile([P, DT, QB * P], FP8, tag=f"xq{s}", name=f"XQ{s}") for s in range(2)]
            XT = [big.tile([P, DT, T], FP8, tag=f"xt{s}", name=f"XT{s}") for s in range(2)]
            XN = [big.tile([P, KVB, D], FP8, tag=f"xn{s}", name=f"XN{s}") for s in range(2)]
            XNB = big.tile([P, 2, D], BF16, tag="xnb")
            WV = [big.tile([P, DT, D], FP8, tag=f"wv{s}", name=f"WVt{s}") for s in range(2)]
            WVB = big.tile([P, DT, D], BF16, tag="wvb")
            QT = [big.tile([P, DT, QB * P], FP8, tag=f"qt{s}", name=f"QTt{s}") for s in range(2)]

            # ---- input DMAs, ordered by first use
            for dp in range(4):
                sl = slice(2 * dp, 2 * dp + 2)
                nc.sync.dma_start(out=WF[0][:, sl, :], in_=wf_hi[:, sl, :])
                nc.sync.dma_start(out=XQ[0][:, sl, :], in_=xq_hi[:, sl, :])
                nc.sync.dma_start(out=WF[1][:, sl, :], in_=wf_lo[:, sl, :])
                nc.sync.dma_start(out=XQ[1][:, sl, :], in_=xq_lo[:, sl, :])
            # xt / xn streamed in 512-token chunks, kv-ascending
            for c in range(4):
                tsl = slice(512 * c, 512 * (c + 1))
                ksl = slice(4 * c, 4 * (c + 1))
                nc.sync.dma_start(out=XT[0][:, :, tsl], in_=xt_hi[:, :, tsl])
                nc.sync.dma_start(out=XT[1][:, :, tsl], in_=xt_lo[:, :, tsl])
                if c == 0:
                    nc.sync.dma_start(out=XNB[:], in_=xn_bf[:, :, :])
                nc.sync.dma_start(out=XN[0][:, ksl, :], in_=xn_hi[:, ksl, :])
                nc.sync.dma_start(out=XN[1][:, ksl, :], in_=xn_lo[:, ksl, :])
                if c == 1:
                    nc.sync.dma_start(out=WVB[:], in_=wv_bf[:, :, :])
                if c == 2:
                    nc.sync.dma_start(out=WV[0][:], in_=wv_hi[:, :, :])
                    nc.sync.dma_start(out=WV[1][:], in_=wv_lo[:, :, :])

            # ---- Phase A: Qt (folded q projection), split into hi/lo fp8
            for qc in range(4):
                qsl = slice(256 * qc, 256 * (qc + 1))
                for dt in range(DT):
                    ps = psA.tile([P, 256], FP32, tag="mm")
                    i = 0
                    for dp in range(4):
                        ksl = slice(2 * dp, 2 * dp + 2)
                        for (w, xx) in ((WF[0], XQ[0]), (WF[0], XQ[1]), (WF[1], XQ[0])):
                            nc.tensor.matmul(ps[:], w[:, ksl, dt * P:(dt + 1) * P],
                                             xx[:, ksl, qsl],
                                             start=(i == 0), stop=(i == 11),
                                             perf_mode=DR)
                            i += 1
                    nc.scalar.copy(QT[0][:, dt, qsl], ps[:])
                    nc.vector.tensor_sub(QT[1][:, dt, qsl], ps[:], QT[0][:, dt, qsl])

            # ---- Phases B/C interleaved across q blocks
            # per-block psum state, allocated lazily at first use
            state = {}

            def emit_scores(j, p):
                """scoresT psum for kv pair p of q block j -> exp -> fp8/bf16."""
                ps = psA.tile([P, 256], FP32, tag="mm", name="ps_sc")
                for half in range(2):
                    kb = 2 * p + half
                    i = 0
                    for dp in range(4):
                        ksl = slice(2 * dp, 2 * dp + 2)
                        for (xx, qq) in ((XT[0], QT[0]), (XT[0], QT[1]), (XT[1], QT[0])):
                            nc.tensor.matmul(
                                ps[:, half * P:(half + 1) * P],
                                xx[:, ksl, kb * P:(kb + 1) * P],
                                qq[:, ksl, j * P:(j + 1) * P],
                                start=(i == 0), stop=(i == 11), perf_mode=DR)
                            i += 1
                return ps

            def emit_exp(j, p, ps, ex):
                nc.scalar.activation(ex[:, 2 * p:2 * p + 2, :].rearrange("p a b -> p (a b)"),
                                     ps[:],
                                     mybir.ActivationFunctionType.Exp,
                                     scale=1.0 / 1024.0, bias=cb[:, j:j + 1])
                if p == j:  # diagonal pair: multiplicative causal/padding mask
                    m = msk8 if j > 0 else mskb
                    nc.vector.tensor_mul(
                        ex[:, 2 * p:2 * p + 2, :].rearrange("p a b -> p (a b)"),
                        ex[:, 2 * p:2 * p + 2, :].rearrange("p a b -> p (a b)"),
                        m[:])

            def emit_sums(j, p, ex):
                """Pair-streamed sums: single open psum group, alone in its
                bank. sums transposed: stationary=exps, moving=ones col."""
                if p == 0:
                    state[('s', j)] = psSum.tile([P, 1], FP32, tag="sums",
                                                 name="sums_ps")
                sums_ps = state[('s', j)]
                if j > 0:
                    nc.tensor.matmul(sums_ps[:], ex[:, 2 * p:2 * p + 2, :],
                                     ones8[:, :, :1],
                                     start=(p == 0), stop=(p == j), perf_mode=DR)
                else:
                    for kb in range(2):
                        nc.tensor.matmul(sums_ps[:], ex[:, kb, :], onesb[:, :1],
                                         start=(kb == 0), stop=(kb == 1))

            def emit_ctx(j, ds, ex):
                """ctxT accumulation for one d-slice over ALL kv pairs of block
                j. ds-outer/pair-inner: PSUM allows only one open accumulation
                group per bank, so each region's group must run uninterrupted."""
                if ds == 0:
                    state[('u', j)] = psU.tile([P, DT, P], FP32, tag="u",
                                               name="u_ps")
                u_ps = state[('u', j)]
                if j > 0:
                    for p in range(j + 1):
                        for s in range(2):
                            nc.tensor.matmul(u_ps[:, ds, :],
                                             XN[s][:, 2 * p:2 * p + 2, ds * P:(ds + 1) * P],
                                             ex[:, 2 * p:2 * p + 2, :],
                                             start=(p == 0 and s == 0),
                                             stop=(p == j and s == 1), perf_mode=DR)
                else:
                    for kb in range(2):
                        nc.tensor.matmul(u_ps[:, ds, :],
                                         XNB[:, kb, ds * P:(ds + 1) * P],
                                         ex[:, kb, :],
                                         start=(kb == 0), stop=(kb == 1))

            def emit_recip_usplit(j):
                sums_ps = state.pop(('s', j))
                u_ps = state.pop(('u', j))
                recip = small.tile([P, 1], FP32, tag="recip")
                nc.vector.reciprocal(recip[:], sums_ps[:])
                if j > 0:
                    uh = upool.tile([P, DT, P], FP8, tag="uh")
                    ul = upool.tile([P, DT, P], FP8, tag="ul")
                    for h in range(2):
                        dsl = slice(4 * h, 4 * h + 4)
                        nc.scalar.copy(uh[:, dsl, :], u_ps[:, dsl, :])
                        nc.vector.tensor_sub(ul[:, dsl, :], u_ps[:, dsl, :], uh[:, dsl, :])
                    state[j] = (uh, ul, recip)
                else:
                    ub = upool.tile([P, DT, P], BF16, tag="ub")
                    nc.scalar.copy(ub[:], u_ps[:])
                    state[j] = (ub, None, recip)

            def emit_wv(j, ec, o_sb):
                usrc, ulo, recip = state[j]
                po = psO.tile([P, 512], FP32, tag="po")
                if j > 0:
                    for half in range(2):
                        esl = slice(512 * ec + 256 * half, 512 * ec + 256 * (half + 1))
                        i = 0
                        for dp in range(4):
                            ksl = slice(2 * dp, 2 * dp + 2)
                            for (uu, ww) in ((usrc, WV[0]), (usrc, WV[1]), (ulo, WV[0])):
                                nc.tensor.matmul(po[:, half * 256:(half + 1) * 256],
                                                 uu[:, ksl, :], ww[:, ksl, esl],
                                                 start=(i == 0), stop=(i == 11),
                                                 perf_mode=DR)
                                i += 1
                else:
                    esl = slice(512 * ec, 512 * (ec + 1))
                    for dt in range(DT):
                        nc.tensor.matmul(po[:], usrc[:, dt, :], WVB[:, dt, esl],
                                         start=(dt == 0), stop=(dt == DT - 1))
                nc.scalar.activation(o_sb[:, 512 * ec:512 * (ec + 1)], po[:],
                                     mybir.ActivationFunctionType.Copy,
                                     scale=recip[:])
                if ec == 1:
                    nc.sync.dma_start(out=out[j * P:(j + 1) * P, :], in_=o_sb[:])
                    del state[j]

            # Deferred PE-work queue: sums/ctx and WvApp chunks are emitted
            # between later score groups so their ACT/DVE deps resolve while
            # the PE stays busy. At most 2 items pop per score slot, and a
            # block's WvApp chunks are interleaved into the NEXT block's
            # score slots so they never chase their own U-split.
            queue = []

            def drain(keep=2, max_pop=2):
                n = 0
                while len(queue) > keep and n < max_pop:
                    queue.pop(0)()
                    n += 1

            for j in range(QB):
                ex = epool.tile([P, max(2 * (j + 1), 4), P],
                                FP8 if j > 0 else BF16, tag=f"ex{j % 2}", name=f"ex{j}")
                for p in range(j + 1):
                    ps = emit_scores(j, p)
                    emit_exp(j, p, ps, ex)
                    queue.append(lambda j=j, p=p, ex=ex: emit_sums(j, p, ex))
                    drain(keep=2, max_pop=3)
                # block j's ctx (all pairs per d-slice), recip/U-split and
                # WvApp drain under block j+1's score groups.
                for ds in range(DT):
                    queue.append(lambda j=j, ds=ds, ex=ex: emit_ctx(j, ds, ex))
                o_sb = opool.tile([P, D], FP32, tag="osb", name=f"osb{j}")
                queue.append(lambda j=j: emit_recip_usplit(j))
                queue.append(lambda j=j, o=o_sb: emit_wv(j, 0, o))
                queue.append(lambda j=j, o=o_sb: emit_wv(j, 1, o))
            while queue:
                queue.pop(0)()

    nc.compile()
    return nc


def _get_nc():
    global _CACHED_NC
    if _CACHED_NC is None:
        _CACHED_NC = _build()
    return _CACHED_NC


def _split8(a):
    hi = np.ascontiguousarray(a).astype(F8)
    lo = (a - hi.astype(np.float32)).astype(F8)
    return hi, lo


def _prep_inputs(x, Wq, Wk, Wv):
    tril = np.tril(np.ones((P, P), np.float32))
    triuT = tril.T.copy()  # mask in [kv, q] layout: pass iff kv <= q
    ones = np.ones((P, P), np.float32)
    zeros = np.zeros((P, P), np.float32)
    wfold = (np.asarray(Wq, np.float64) @ np.asarray(Wk, np.float64).T)
    wf32 = (wfold * 32.0).astype(np.float32)
    wv32 = np.asarray(Wv, np.float32) * 32.0
    wf_hi, wf_lo = _split8(wf32.reshape(DT, P, D).transpose(1, 0, 2))
    wv_hi, wv_lo = _split8(wv32.reshape(DT, P, D).transpose(1, 0, 2))
    wv_bf = np.ascontiguousarray(wv32.reshape(DT, P, D).transpose(1, 0, 2)).astype(BF)
    cb = np.zeros((P, QB), np.float32)
    for j in range(QB):
        cb[:, j] = -(1.5 + np.log(j + 1.0))
    in_maps = []
    for core in range(NCORES):
        b, r = core // 2, core % 2
        xb = np.asarray(x[b], np.float32)            # [T, D]
        xtv = xb.T.reshape(DT, P, T).transpose(1, 0, 2)    # [P, DT, T]
        xt_hi, xt_lo = _split8(xtv)
        xqv = np.ascontiguousarray(
            xtv.reshape(P, DT, KVB, P)[:, :, r::2, :].reshape(P, DT, QB * P))
        xq_hi, xq_lo = _split8(xqv)
        xnv = xb.reshape(KVB, P, D).transpose(1, 0, 2)     # [P, KVB, D]
        xn_hi, xn_lo = _split8(xnv)
        xn_bf = np.ascontiguousarray(xnv[:, :2, :]).astype(BF)
        m = (np.concatenate([triuT, zeros], axis=1) if r == 0
             else np.concatenate([ones, triuT], axis=1))
        in_maps.append({
            "xq_hi": xq_hi, "xq_lo": xq_lo,
            "xt_hi": xt_hi, "xt_lo": xt_lo,
            "xn_hi": xn_hi, "xn_lo": xn_lo, "xn_bf": xn_bf,
            "wf_hi": wf_hi, "wf_lo": wf_lo,
            "wv_hi": wv_hi, "wv_lo": wv_lo, "wv_bf": wv_bf,
            "mask8": m.astype(F8), "maskb": m.astype(BF),
            "cbias": cb,
        })
    return in_maps


def _assemble(results, x_shape):
    outp = np.empty(x_shape, np.float32)
    for core in range(NCORES):
        b, r = core // 2, core % 2
        co = results[core]["out"]
        for j in range(QB):
            g = 2 * j + r
            outp[b, g * P:(g + 1) * P, :] = co[j * P:(j + 1) * P, :]
    return outp


def kernel(x, Wq, Wk, Wv):
    assert x.shape == (B, T, D) and Wq.shape == (D, D)
    nc = _get_nc()
    in_maps = _prep_inputs(x, Wq, Wk, Wv)
    res = run_bass_kernel_spmd(nc, in_maps, core_ids=list(range(NCORES)))
    return _assemble(res.results, x.shape)
